# revision 7
# baseline (speedup 1.0000x reference)
"""Conv2d(256->256, 3x3, pad 1) via 1-D Winograd F(4,3) along W, direct 3-tap
accumulation along H; H-sharded over 8 TRN2 cores (64 output rows each).

Host precomputes the B^T input transform (6 phase planes z[j][ci, h, 128] in
fp16, 1.5x data inflation) and the G weight transform U[kh, j][ci, co]. Each
output-row group of 4 rows needs, per (j, co-chunk), one PSUM accumulation of
6 fp16 matmuls (3 kh x 2 ci-chunks) with 512-wide moving operands (4 rows x
128 tiles) -- 1152 matmuls/core = 246 us of PE vs 492 us for direct conv.
PSUM drains to fp16 SBUF split across Act (j=0..3), DVE (j=4), GpSimd (j=5);
the A^T output transform runs on DVE in 2x fp16 mode; output is written as
phase-separated fp16 [cc, co, h, p, t] and the host interleaves + casts.
"""

import hashlib
import os
import shutil
import threading

import numpy as np

import concourse.bacc as bacc
import concourse.bass2jax as bass2jax
import concourse.tile as tile
from concourse import mybir
from concourse.bass_utils import run_bass_kernel_spmd

f32 = mybir.dt.float32
f16 = mybir.dt.float16
ADD = mybir.AluOpType.add
SUB = mybir.AluOpType.subtract
MUL = mybir.AluOpType.mult

_NEFF_CACHE = os.path.join(os.path.expanduser("~"), ".bass-neff-cache")


def _install_neff_cache():
    orig = getattr(bass2jax, "compile_bir_kernel", None)
    if orig is None or getattr(orig, "_neff_cached", False):
        return

    def cached(bir_json, tmpdir, neff_name="file.neff"):
        cpath = None
        try:
            raw = bir_json if isinstance(bir_json, bytes) else bir_json.encode()
            raw = raw.replace(os.path.abspath(__file__).encode(), b"@KERNEL@")
            cpath = os.path.join(_NEFF_CACHE,
                                 hashlib.sha256(raw).hexdigest() + ".neff")
            if os.path.exists(cpath):
                dst = os.path.join(tmpdir, neff_name)
                shutil.copyfile(cpath, dst)
                return dst
        except Exception:
            cpath = None
        out = orig(bir_json, tmpdir, neff_name)
        if cpath:
            try:
                os.makedirs(_NEFF_CACHE, exist_ok=True)
                tmp = f"{cpath}.tmp{os.getpid()}"
                shutil.copyfile(out, tmp)
                os.replace(tmp, cpath)
            except Exception:
                pass
        return out

    cached._neff_cached = True
    bass2jax.compile_bir_kernel = cached


_install_neff_cache()


def _in_clean_thread(fn):
    res = {}

    def runner():
        try:
            res["v"] = fn()
        except BaseException as e:
            res["e"] = e

    t = threading.Thread(target=runner, name="convkernel")
    t.start()
    t.join()
    if "e" in res:
        raise res["e"]
    return res["v"]


NCORES = 8
CIN = 256
COUT = 256
H = 512
W = 512
PC = 128
HB = H // NCORES          # 64 output rows per core
HIN = HB + 2              # 66 padded input rows per core
NJ = 6                    # F(4,3) transform size
NT = W // 4               # 128 output tiles of 4 per row
NSTRIP = 4                # row strips per core
GPS = 4                   # groups per strip
RG = 4                    # output rows per group
SROWS = GPS * RG + 2      # 18 input rows per strip

BT = np.array([
    [4, 0, -5, 0, 1, 0],
    [0, -4, -4, 1, 1, 0],
    [0, 4, -4, -1, 1, 0],
    [0, -2, -1, 2, 1, 0],
    [0, 2, -1, -2, 1, 0],
    [0, 4, 0, -5, 0, 1]], dtype=np.float32)
G = np.array([
    [1 / 4, 0, 0],
    [-1 / 6, -1 / 6, -1 / 6],
    [-1 / 6, 1 / 6, -1 / 6],
    [1 / 24, 1 / 12, 1 / 6],
    [1 / 24, -1 / 12, 1 / 6],
    [0, 0, 1]], dtype=np.float64)

_nc_cache = {}


def _build(repeats=1):
    nc = bacc.Bacc("TRN2", target_bir_lowering=False, debug=False,
                   num_devices=NCORES)
    zs = nc.dram_tensor("zs", [PC, 2, NJ, HIN, NT], f16,
                        kind="ExternalInput").ap()
    ut = nc.dram_tensor("ut", [PC, 2, NJ, 2, 3, PC], f16,
                        kind="ExternalInput").ap()
    out = nc.dram_tensor("out", [2, PC, HB, 4, NT], f16,
                         kind="ExternalOutput").ap()

    with tile.TileContext(nc) as tc:
        with tc.tile_pool(name="wpool", bufs=1) as wpool, \
             tc.tile_pool(name="zpool", bufs=6) as zpool, \
             tc.tile_pool(name="mpool", bufs=2) as mpool, \
             tc.tile_pool(name="tpool", bufs=2) as tpool, \
             tc.tile_pool(name="ypool", bufs=2) as ypool, \
             tc.tile_pool(name="pspool", bufs=8, space="PSUM") as pspool:

            # Warm the PE clock gate while input DMAs are in flight.
            warm_src = wpool.tile([PC, PC], f16, name="warm_src")
            nc.vector.memset(warm_src[:], 0.0)
            warm_ps = pspool.tile([PC, PC], f32, tag="ps", name="warm_ps")
            for i in range(12):
                nc.tensor.matmul(warm_ps[:], warm_src[:], warm_src[:],
                                 start=True, stop=True)

            u_all = wpool.tile([PC, 2, NJ, 2, 3, PC], f16, name="u_all")
            nc.sync.dma_start(u_all[:, 0, 0, :, :, :], ut[:, 0, 0, :, :, :])

            def conv_chunk(z_t, lr, h0, co, rows, sfx):
                pss = []
                for j in range(NJ):
                    ps = pspool.tile([PC, rows * NT], f32, tag="ps",
                                     name=f"ps_{sfx}_{j}")
                    pss.append(ps)
                    idx = 0
                    for kh in range(3):
                        for ci in range(2):
                            nc.tensor.matmul(
                                ps[:],
                                u_all[:, co, j, ci, kh, :],
                                z_t[:, ci, j, lr + kh:lr + kh + rows, :],
                                start=(idx == 0), stop=(idx == 5))
                            idx += 1
                m_t = mpool.tile([PC, NJ, rows, NT], f16,
                                 tag=f"m{co}_{rows}", name=f"m_{sfx}")
                for j in range(NJ):
                    nc.scalar.copy(m_t[:, j, :, :], pss[j][:])
                # A^T output transform on DVE (fp16, 2x mode)
                tt = tpool.tile([PC, 5, rows, NT], f16,
                                tag=f"t{co}_{rows}", name=f"t_{sfx}")
                y_t = ypool.tile([PC, rows, 4, NT], f16,
                                 tag=f"y{co}_{rows}", name=f"y_{sfx}")
                m = [m_t[:, j, :, :] for j in range(NJ)]
                tm = tt[:, 0, :, :]   # m1 - m2
                sm = tt[:, 1, :, :]   # m1 + m2
                um = tt[:, 2, :, :]   # m3 - m4
                vm = tt[:, 3, :, :]   # m3 + m4
                p0a = tt[:, 4, :, :]  # sm + vm
                nc.vector.tensor_tensor(tm, m[1], m[2], SUB)
                nc.vector.tensor_tensor(sm, m[1], m[2], ADD)
                nc.vector.tensor_tensor(um, m[3], m[4], SUB)
                nc.vector.tensor_tensor(vm, m[3], m[4], ADD)
                nc.vector.tensor_tensor(p0a, sm, vm, ADD)
                nc.vector.tensor_tensor(y_t[:, :, 0, :], p0a, m[0], ADD)
                nc.vector.scalar_tensor_tensor(
                    y_t[:, :, 1, :], um, 2.0, tm, MUL, ADD)
                nc.vector.scalar_tensor_tensor(
                    y_t[:, :, 2, :], vm, 4.0, sm, MUL, ADD)
                nc.vector.scalar_tensor_tensor(p0a, um, 8.0, tm, MUL, ADD)
                nc.vector.tensor_tensor(y_t[:, :, 3, :], p0a, m[5], ADD)
                nc.sync.dma_start(out[co, :, h0:h0 + rows, :, :], y_t[:])

            NG = NSTRIP * GPS                           # 16 groups of 4 rows
            for g in range(NG):
                z_t = zpool.tile([PC, 2, NJ, RG + 2, NT], f16,
                                 tag="z", name=f"z_{g}")
                r0 = g * RG
                if g == 0:
                    # Per-phase pieces so the first matmul (j=0) gates on
                    # ~0.4 MB instead of the whole 2.4 MB piece.
                    for j in range(NJ):
                        nc.sync.dma_start(
                            z_t[:, :, j, :, :],
                            zs[:, :, j, r0:r0 + RG + 2, :])
                    for co in range(2):
                        for j in range(NJ):
                            if co == 0 and j == 0:
                                continue
                            nc.sync.dma_start(
                                u_all[:, co, j, :, :, :],
                                ut[:, co, j, :, :, :])
                else:
                    nc.sync.dma_start(z_t[:], zs[:, :, :, r0:r0 + RG + 2, :])
                for _rep in range(repeats):
                    for co in range(2):
                        if g == NG - 1 and co == 1:
                            # Halve the tail: last chunk in two 2-row pieces.
                            half = RG // 2
                            conv_chunk(z_t, 0, g * RG, co, half, f"{g}_{co}a")
                            conv_chunk(z_t, half, g * RG + half, co, half,
                                       f"{g}_{co}b")
                        else:
                            conv_chunk(z_t, 0, g * RG, co, RG, f"{g}_{co}")
    nc.compile()
    return nc


def _get_nc(repeats=1):
    if repeats not in _nc_cache:
        _nc_cache[repeats] = _in_clean_thread(lambda: _build(repeats))
    return _nc_cache[repeats]


def _make_in_maps(x, weight):
    # x: [1, 256, 512, 512] fp32; weight: [256, 256, 3, 3] fp32
    x_pad = np.zeros((PC, 2, H + 2, W + 2), dtype=np.float32)
    x_pad[:, :, 1:H + 1, 1:W + 1] = x[0].reshape(2, PC, H, W).transpose(
        1, 0, 2, 3)
    # z[j] = sum_k BT[j,k] * x_pad[..., 4t+k]  -> [128, 2, 514, 6, 128] fp16
    z_full = np.empty((PC, 2, NJ, H + 2, NT), dtype=np.float16)
    dk = [x_pad[:, :, :, k:k + 4 * NT:4] for k in range(6)]
    for j in range(NJ):
        acc = None
        for k in range(6):
            c = BT[j, k]
            if c == 0:
                continue
            term = dk[k] if c == 1 else dk[k] * c
            acc = term.copy() if acc is None else acc + term
        z_full[:, :, j, :, :] = acc.astype(np.float16)
    # U[j,o,i,kh] = sum_kw G[j,kw] w[o,i,kh,kw] -> [128, 2, 3, 6, 2, 128] fp16
    # device layout [ci_in, co_cc, j, ci_cc, kh, co_in]
    U = np.einsum("jw,oihw->ihjo", G, weight.astype(np.float64))
    # U[i, h, j, o] -> [i_in(128), o_cc(2), j, i_cc(2), h(3), o_in(128)]
    U = U.reshape(2, PC, 3, NJ, 2, PC)            # [i_cc, i_in, h, j, o_cc, o_in]
    U = U.transpose(1, 4, 3, 0, 2, 5)             # [i_in, o_cc, j, i_cc, h, o_in]
    U = np.ascontiguousarray(U, dtype=np.float16)
    in_maps = []
    for core in range(NCORES):
        r0 = core * HB
        in_maps.append(
            {"zs": z_full[:, :, :, r0:r0 + HIN, :], "ut": U})
    return in_maps


def kernel(x, weight):
    x = np.asarray(x, dtype=np.float32)
    weight = np.asarray(weight, dtype=np.float32)
    nc = _get_nc(1)
    in_maps = _make_in_maps(x, weight)
    res = _in_clean_thread(lambda: run_bass_kernel_spmd(
        nc, in_maps, core_ids=list(range(NCORES))))
    parts = []
    for c in range(NCORES):
        arr = res.results[c]["out"]           # [2, 128, 64, 4, 128] f16
        parts.append(arr.transpose(0, 1, 2, 4, 3).reshape(COUT, HB, W))
    full = np.concatenate(parts, axis=1)      # [256, 512, 512]
    return full[None].astype(np.float32)


# revision 8
# speedup vs baseline: 1.0034x; 1.0034x over previous
"""Conv2d(256->256, 3x3, pad 1) via 1-D Winograd F(4,3) along W, direct 3-tap
accumulation along H; H-sharded over 8 TRN2 cores (64 output rows each).

Host precomputes the B^T input transform (6 phase planes z[j][ci, h, 128] in
fp16, 1.5x data inflation) and the G weight transform U[kh, j][ci, co]. Each
output-row group of 4 rows needs, per (j, co-chunk), one PSUM accumulation of
6 fp16 matmuls (3 kh x 2 ci-chunks) with 512-wide moving operands (4 rows x
128 tiles) -- 1152 matmuls/core = 246 us of PE vs 492 us for direct conv.
PSUM drains to fp16 SBUF split across Act (j=0..3), DVE (j=4), GpSimd (j=5);
the A^T output transform runs on DVE in 2x fp16 mode; output is written as
phase-separated fp16 [cc, co, h, p, t] and the host interleaves + casts.
"""

import hashlib
import os
import shutil
import threading

import numpy as np

import concourse.bacc as bacc
import concourse.bass2jax as bass2jax
import concourse.tile as tile
from concourse import mybir
from concourse.bass_utils import run_bass_kernel_spmd

f32 = mybir.dt.float32
f16 = mybir.dt.float16
ADD = mybir.AluOpType.add
SUB = mybir.AluOpType.subtract
MUL = mybir.AluOpType.mult

_NEFF_CACHE = os.path.join(os.path.expanduser("~"), ".bass-neff-cache")


def _install_neff_cache():
    orig = getattr(bass2jax, "compile_bir_kernel", None)
    if orig is None or getattr(orig, "_neff_cached", False):
        return

    def cached(bir_json, tmpdir, neff_name="file.neff"):
        cpath = None
        try:
            raw = bir_json if isinstance(bir_json, bytes) else bir_json.encode()
            raw = raw.replace(os.path.abspath(__file__).encode(), b"@KERNEL@")
            cpath = os.path.join(_NEFF_CACHE,
                                 hashlib.sha256(raw).hexdigest() + ".neff")
            if os.path.exists(cpath):
                dst = os.path.join(tmpdir, neff_name)
                shutil.copyfile(cpath, dst)
                return dst
        except Exception:
            cpath = None
        out = orig(bir_json, tmpdir, neff_name)
        if cpath:
            try:
                os.makedirs(_NEFF_CACHE, exist_ok=True)
                tmp = f"{cpath}.tmp{os.getpid()}"
                shutil.copyfile(out, tmp)
                os.replace(tmp, cpath)
            except Exception:
                pass
        return out

    cached._neff_cached = True
    bass2jax.compile_bir_kernel = cached


_install_neff_cache()


def _in_clean_thread(fn):
    res = {}

    def runner():
        try:
            res["v"] = fn()
        except BaseException as e:
            res["e"] = e

    t = threading.Thread(target=runner, name="convkernel")
    t.start()
    t.join()
    if "e" in res:
        raise res["e"]
    return res["v"]


NCORES = 8
CIN = 256
COUT = 256
H = 512
W = 512
PC = 128
HB = H // NCORES          # 64 output rows per core
HIN = HB + 2              # 66 padded input rows per core
NJ = 6                    # F(4,3) transform size
NT = W // 4               # 128 output tiles of 4 per row
NSTRIP = 4                # row strips per core
GPS = 4                   # groups per strip
RG = 4                    # output rows per group
SROWS = GPS * RG + 2      # 18 input rows per strip

BT = np.array([
    [4, 0, -5, 0, 1, 0],
    [0, -4, -4, 1, 1, 0],
    [0, 4, -4, -1, 1, 0],
    [0, -2, -1, 2, 1, 0],
    [0, 2, -1, -2, 1, 0],
    [0, 4, 0, -5, 0, 1]], dtype=np.float32)
G = np.array([
    [1 / 4, 0, 0],
    [-1 / 6, -1 / 6, -1 / 6],
    [-1 / 6, 1 / 6, -1 / 6],
    [1 / 24, 1 / 12, 1 / 6],
    [1 / 24, -1 / 12, 1 / 6],
    [0, 0, 1]], dtype=np.float64)

_nc_cache = {}


def _build(repeats=1):
    nc = bacc.Bacc("TRN2", target_bir_lowering=False, debug=False,
                   num_devices=NCORES)
    zs = nc.dram_tensor("zs", [PC, 2, NJ, HIN, NT], f16,
                        kind="ExternalInput").ap()
    ut = nc.dram_tensor("ut", [PC, 2, NJ, 2, 3, PC], f16,
                        kind="ExternalInput").ap()
    out = nc.dram_tensor("out", [2, PC, HB, 4, NT], f16,
                         kind="ExternalOutput").ap()

    with tile.TileContext(nc) as tc:
        with tc.tile_pool(name="wpool", bufs=1) as wpool, \
             tc.tile_pool(name="zpool", bufs=6) as zpool, \
             tc.tile_pool(name="mpool", bufs=2) as mpool, \
             tc.tile_pool(name="tpool", bufs=2) as tpool, \
             tc.tile_pool(name="ypool", bufs=2) as ypool, \
             tc.tile_pool(name="pspool", bufs=8, space="PSUM") as pspool:

            # Warm the PE clock gate while input DMAs are in flight.
            warm_src = wpool.tile([PC, PC], f16, name="warm_src")
            nc.gpsimd.memset(warm_src[:], 0.0)
            warm_ps = pspool.tile([PC, PC], f32, tag="ps", name="warm_ps")
            for i in range(12):
                nc.tensor.matmul(warm_ps[:], warm_src[:], warm_src[:],
                                 start=True, stop=True)

            u_all = wpool.tile([PC, 2, NJ, 2, 3, PC], f16, name="u_all")
            nc.sync.dma_start(u_all[:, 0, 0, :, :, :], ut[:, 0, 0, :, :, :])

            def conv_chunk(z_t, lr, h0, co, rows, sfx, fast_tail=False):
                pss = []
                for j in range(NJ):
                    ps = pspool.tile([PC, rows * NT], f32, tag="ps",
                                     name=f"ps_{sfx}_{j}")
                    pss.append(ps)
                    idx = 0
                    for kh in range(3):
                        for ci in range(2):
                            nc.tensor.matmul(
                                ps[:],
                                u_all[:, co, j, ci, kh, :],
                                z_t[:, ci, j, lr + kh:lr + kh + rows, :],
                                start=(idx == 0), stop=(idx == 5))
                            idx += 1
                m_t = mpool.tile([PC, NJ, rows, NT], f16,
                                 tag=f"m{co}_{rows}", name=f"m_{sfx}")
                for j in range(NJ):
                    if fast_tail and j % 2 == 1:
                        nc.vector.tensor_copy(m_t[:, j, :, :], pss[j][:])
                    else:
                        nc.scalar.copy(m_t[:, j, :, :], pss[j][:])
                # A^T output transform on DVE (fp16, 2x mode)
                tt = tpool.tile([PC, 5, rows, NT], f16,
                                tag=f"t{co}_{rows}", name=f"t_{sfx}")
                y_t = ypool.tile([PC, rows, 4, NT], f16,
                                 tag=f"y{co}_{rows}", name=f"y_{sfx}")
                m = [m_t[:, j, :, :] for j in range(NJ)]
                tm = tt[:, 0, :, :]   # m1 - m2
                sm = tt[:, 1, :, :]   # m1 + m2
                um = tt[:, 2, :, :]   # m3 - m4
                vm = tt[:, 3, :, :]   # m3 + m4
                p0a = tt[:, 4, :, :]  # sm + vm
                nc.vector.tensor_tensor(tm, m[1], m[2], SUB)
                nc.vector.tensor_tensor(sm, m[1], m[2], ADD)
                nc.vector.tensor_tensor(um, m[3], m[4], SUB)
                nc.vector.tensor_tensor(vm, m[3], m[4], ADD)
                nc.vector.tensor_tensor(p0a, sm, vm, ADD)
                nc.vector.tensor_tensor(y_t[:, :, 0, :], p0a, m[0], ADD)
                nc.vector.scalar_tensor_tensor(
                    y_t[:, :, 1, :], um, 2.0, tm, MUL, ADD)
                nc.vector.scalar_tensor_tensor(
                    y_t[:, :, 2, :], vm, 4.0, sm, MUL, ADD)
                nc.vector.scalar_tensor_tensor(p0a, um, 8.0, tm, MUL, ADD)
                nc.vector.tensor_tensor(y_t[:, :, 3, :], p0a, m[5], ADD)
                nc.sync.dma_start(out[co, :, h0:h0 + rows, :, :], y_t[:])

            NG = NSTRIP * GPS                           # 16 groups of 4 rows
            for g in range(NG):
                z_t = zpool.tile([PC, 2, NJ, RG + 2, NT], f16,
                                 tag="z", name=f"z_{g}")
                r0 = g * RG
                if g == 0:
                    # Per-phase pieces so the first matmul (j=0) gates on
                    # ~0.4 MB instead of the whole 2.4 MB piece.
                    for j in range(NJ):
                        nc.scalar.dma_start(
                            z_t[:, :, j, :, :],
                            zs[:, :, j, r0:r0 + RG + 2, :])
                    for co in range(2):
                        for j in range(NJ):
                            if co == 0 and j == 0:
                                continue
                            nc.sync.dma_start(
                                u_all[:, co, j, :, :, :],
                                ut[:, co, j, :, :, :])
                else:
                    nc.scalar.dma_start(z_t[:], zs[:, :, :, r0:r0 + RG + 2, :])
                for _rep in range(repeats):
                    for co in range(2):
                        if g == NG - 1 and co == 1:
                            # Halve the tail: last chunk in two 2-row pieces.
                            half = RG // 2
                            conv_chunk(z_t, 0, g * RG, co, half,
                                       f"{g}_{co}a", fast_tail=True)
                            conv_chunk(z_t, half, g * RG + half, co, half,
                                       f"{g}_{co}b", fast_tail=True)
                        else:
                            conv_chunk(z_t, 0, g * RG, co, RG, f"{g}_{co}")
    nc.compile()
    return nc


def _get_nc(repeats=1):
    if repeats not in _nc_cache:
        _nc_cache[repeats] = _in_clean_thread(lambda: _build(repeats))
    return _nc_cache[repeats]


def _make_in_maps(x, weight):
    # x: [1, 256, 512, 512] fp32; weight: [256, 256, 3, 3] fp32
    x_pad = np.zeros((PC, 2, H + 2, W + 2), dtype=np.float32)
    x_pad[:, :, 1:H + 1, 1:W + 1] = x[0].reshape(2, PC, H, W).transpose(
        1, 0, 2, 3)
    # z[j] = sum_k BT[j,k] * x_pad[..., 4t+k]  -> [128, 2, 514, 6, 128] fp16
    z_full = np.empty((PC, 2, NJ, H + 2, NT), dtype=np.float16)
    dk = [x_pad[:, :, :, k:k + 4 * NT:4] for k in range(6)]
    for j in range(NJ):
        acc = None
        for k in range(6):
            c = BT[j, k]
            if c == 0:
                continue
            term = dk[k] if c == 1 else dk[k] * c
            acc = term.copy() if acc is None else acc + term
        z_full[:, :, j, :, :] = acc.astype(np.float16)
    # U[j,o,i,kh] = sum_kw G[j,kw] w[o,i,kh,kw] -> [128, 2, 3, 6, 2, 128] fp16
    # device layout [ci_in, co_cc, j, ci_cc, kh, co_in]
    U = np.einsum("jw,oihw->ihjo", G, weight.astype(np.float64))
    # U[i, h, j, o] -> [i_in(128), o_cc(2), j, i_cc(2), h(3), o_in(128)]
    U = U.reshape(2, PC, 3, NJ, 2, PC)            # [i_cc, i_in, h, j, o_cc, o_in]
    U = U.transpose(1, 4, 3, 0, 2, 5)             # [i_in, o_cc, j, i_cc, h, o_in]
    U = np.ascontiguousarray(U, dtype=np.float16)
    in_maps = []
    for core in range(NCORES):
        r0 = core * HB
        in_maps.append(
            {"zs": z_full[:, :, :, r0:r0 + HIN, :], "ut": U})
    return in_maps


def kernel(x, weight):
    x = np.asarray(x, dtype=np.float32)
    weight = np.asarray(weight, dtype=np.float32)
    nc = _get_nc(1)
    in_maps = _make_in_maps(x, weight)
    res = _in_clean_thread(lambda: run_bass_kernel_spmd(
        nc, in_maps, core_ids=list(range(NCORES))))
    parts = []
    for c in range(NCORES):
        arr = res.results[c]["out"]           # [2, 128, 64, 4, 128] f16
        parts.append(arr.transpose(0, 1, 2, 4, 3).reshape(COUT, HB, W))
    full = np.concatenate(parts, axis=1)      # [256, 512, 512]
    return full[None].astype(np.float32)
